# revision 1
# baseline (speedup 1.0000x reference)
"""Bass/Trainium2 kernel for nn_CustomConvWithExtra.

Reference computation (B=32, CIN=COUT=64, H=W=128, K=3, FES=3):
  main = conv3x3(x, conv_w, pad=1) + conv_b
  extra = grouped_conv3x3(broadcast(extra_inputs), extra_w, pad=1) + extra_b
  out = main + extra

Key observation: the "extra" path's input is spatially constant per
(sample, channel), so its conv collapses to 9 border-case scalars per
(sample, cout) (interior / 4 edges / 4 corners).  Those scalars (plus
conv_b + extra_b) are precomputed on the host and folded into the
PSUM->SBUF epilogue as a positional add-map.  The device does the real
work: the dense 3x3 conv as 9 shifted fp32 matmuls accumulating in PSUM.

Sharding: data-parallel over batch, 4 samples per core x 8 cores.
Each core processes its samples in 2 "sample pairs": sample 2p on SBUF
partitions 0-63, sample 2p+1 on partitions 64-127.  The 9 tap matmuls
run as concurrent diagonal-quadrant pairs (tile_position (0,0)/(64,64)),
so both samples' bands compute simultaneously in the 128x128 PE array.

SBUF x layout (per sample, partitions = CIN): padded rows of stride 129
= [128 cols][1 zero pad], with a zero halo row above and below and one
extra leading zero.  x[row, col] lives at free offset 1 + (row+1)*129 +
col.  Under this layout every conv tap (di, dj) for an output band
starting at row i0 is a contiguous rhs window at offset
(i0+di)*129 + dj, and all image-border zeros fall out automatically.
"""

import numpy as np

import concourse.bass as bass
import concourse.mybir as mybir
from concourse.tile import TileContext
from concourse.bass_utils import run_bass_kernel_spmd

N_CORES = 8
B, CIN, COUT, FES, H, W, KK = 32, 64, 64, 3, 128, 128, 3
BL = B // N_CORES          # samples per core
NPAIR = BL // 2            # sample pairs per core
RSTRIDE = 129              # padded row stride (W + 1 pad col)
XFREE = 1 + (H + 2) * RSTRIDE + 3   # 16774: lead zero + 130 padded rows + tail pad
RB = 3                     # output rows per band (PSUM tile)
NBAND = (H + RB - 1) // RB  # 43 bands; last band has 2 rows
NMAX = RB * RSTRIDE        # 387 fp32 <= 512 (one PSUM bank)
EOFFS = None               # computed below


def _band_rows(b):
    i0 = b * RB
    return i0, min(RB, H - i0)


# eadd free-dim offsets: band 0 -> first pattern, 1..41 -> mid, 42 -> last
_E_FIRST, _E_MID, _E_LAST = 0, NMAX, 2 * NMAX
EADD_FREE = 2 * NMAX + (H - RB * (NBAND - 1)) * RSTRIDE  # 387+387+258 = 1032


def split_sync_waits(nc):
    """This toolchain's walrus accepts only ONE sync-wait per instruction.
    Hoist extra waits onto single-wait NoOps inserted just before, on the
    same engine (same queue => same semantics)."""
    for func in nc.m.functions:
        for block in func.blocks:
            out = []
            changed = False
            for inst in block.instructions:
                si = inst.sync_info
                waits = list(si.on_wait) if (si and si.on_wait) else []
                if len(waits) > 1:
                    changed = True
                    for k, w in enumerate(waits[:-1]):
                        nop = mybir.InstNoOp(
                            name=f"{inst.name}-sw{k}",
                            engine=inst.engine,
                            sync_info=mybir.SyncInfo(on_wait=[w], on_update=[]),
                            bass_nofuse=True,
                        )
                        nc.register_instruction(nop, overwrite=True)
                        out.append(nop)
                    inst.sync_info = mybir.SyncInfo(
                        on_wait=[waits[-1]], on_update=list(si.on_update or [])
                    )
                out.append(inst)
            if changed:
                block.instructions = out


def build_program():
    f32 = mybir.dt.float32
    nc = bass.Bass("TRN2", target_bir_lowering=False, debug=False,
                   num_devices=N_CORES)
    x = nc.dram_tensor("x", [BL, CIN, H, W], f32, kind="ExternalInput")
    wt = nc.dram_tensor("wt", [128, 9 * COUT], f32, kind="ExternalInput")
    eadd = nc.dram_tensor("eadd", [NPAIR, 128, EADD_FREE], f32,
                          kind="ExternalInput")
    out = nc.dram_tensor("out", [BL, COUT, H, W], f32, kind="ExternalOutput")

    with TileContext(nc) as tc:
        with (
            tc.tile_pool(name="wp", bufs=1) as wp,
            tc.tile_pool(name="xp", bufs=2) as xp,
            tc.tile_pool(name="ep", bufs=2) as ep,
            tc.tile_pool(name="op", bufs=4) as op,
            tc.tile_pool(name="pp", bufs=8, space="PSUM") as pp,
        ):
            wt_sb = wp.tile([128, 9 * COUT], f32)
            nc.sync.dma_start(out=wt_sb[:], in_=wt[:])

            for sp in range(NPAIR):
                xt = xp.tile([128, XFREE], f32, tag="xt")
                # zero the pad structure: leading zero + top halo row (+pad),
                # bottom halo row + tail, and the per-row pad column.
                nc.vector.memset(xt[:, 0:1 + RSTRIDE], 0.0)
                tail = 1 + (H + 1) * RSTRIDE
                nc.vector.memset(xt[:, tail:XFREE], 0.0)
                pads = xt[:, 1 + RSTRIDE:1 + (H + 1) * RSTRIDE].rearrange(
                    "p (r c) -> p r c", c=RSTRIDE)[:, :, W:W + 1]
                nc.vector.memset(pads, 0.0)
                # load the two samples into the two partition halves
                for hhalf in range(2):
                    dst = xt[hhalf * 64:(hhalf + 1) * 64,
                             1 + RSTRIDE:1 + (H + 1) * RSTRIDE].rearrange(
                        "p (r c) -> p r c", c=RSTRIDE)[:, :, 0:W]
                    nc.sync.dma_start(out=dst, in_=x[2 * sp + hhalf])
                et = ep.tile([128, EADD_FREE], f32, tag="et")
                nc.sync.dma_start(out=et[:], in_=eadd[sp])

                for b in range(NBAND):
                    i0, rb = _band_rows(b)
                    n = rb * RSTRIDE
                    ps = pp.tile([128, NMAX], f32, tag="ps")
                    for tap in range(9):
                        di, dj = divmod(tap, 3)
                        off = (i0 + di) * RSTRIDE + dj
                        st, sp_ = (tap == 0), (tap == 8)
                        nc.tensor.matmul(
                            ps[0:64, 0:n],
                            wt_sb[0:64, tap * COUT:(tap + 1) * COUT],
                            xt[0:64, off:off + n], start=st, stop=sp_)
                        nc.tensor.matmul(
                            ps[64:128, 0:n],
                            wt_sb[64:128, tap * COUT:(tap + 1) * COUT],
                            xt[64:128, off:off + n], start=st, stop=sp_)
                    eo = _E_FIRST if b == 0 else (_E_LAST if b == NBAND - 1
                                                  else _E_MID)
                    ot = op.tile([128, NMAX], f32, tag="ot")
                    nc.vector.tensor_add(ot[:, 0:n], ps[:, 0:n],
                                         et[:, eo:eo + n])
                    for hhalf in range(2):
                        src = ot[hhalf * 64:(hhalf + 1) * 64, 0:n].rearrange(
                            "p (r c) -> p r c", c=RSTRIDE)[:, :, 0:W]
                        nc.sync.dma_start(
                            out=out[2 * sp + hhalf, :, i0:i0 + rb, :], in_=src)

    split_sync_waits(nc)
    return nc


_PROGRAM = None


def _get_program():
    global _PROGRAM
    if _PROGRAM is None:
        _PROGRAM = build_program()
    return _PROGRAM


def host_prepack(extra_inputs, conv_w, conv_b, extra_w, extra_b):
    """Fold weights/biases/extra-path into device-ready arrays."""
    # wt[ci, tap*64+co] = conv_w[co, ci, di, dj], tap = di*3+dj; both halves
    wt_half = np.ascontiguousarray(
        conv_w.transpose(1, 2, 3, 0)).reshape(CIN, 9 * COUT)
    wt = np.concatenate([wt_half, wt_half], axis=0).astype(np.float32)

    # border-case extra values: E[s, rowclass, colclass, co]
    row_sel = [slice(1, 3), slice(0, 3), slice(0, 2)]   # top, mid, bot
    col_sel = [slice(1, 3), slice(0, 3), slice(0, 2)]   # left, mid, right
    wsum = np.zeros((3, 3, COUT, FES), np.float32)
    for rc in range(3):
        for cc in range(3):
            wsum[rc, cc] = extra_w[:, :, row_sel[rc], col_sel[cc]].sum((2, 3))
    ein = extra_inputs.reshape(B, COUT, FES)
    e9 = np.einsum('scf,rkcf->srkc', ein, wsum)
    e9 = e9 + (extra_b + conv_b)[None, None, None, :]   # [s, rc, cc, co]

    # positional row patterns at stride 129 (last slot = pad, value 0)
    def row_vec(s, rc):
        v = np.zeros((COUT, RSTRIDE), np.float32)
        v[:, 0] = e9[s, rc, 0]
        v[:, 1:W - 1] = e9[s, rc, 1][:, None]
        v[:, W - 1] = e9[s, rc, 2]
        return v

    eadd = np.zeros((B, COUT, EADD_FREE), np.float32)
    for s in range(B):
        top, mid, bot = row_vec(s, 0), row_vec(s, 1), row_vec(s, 2)
        eadd[s, :, 0:NMAX] = np.concatenate([top, mid, mid], 1)
        eadd[s, :, NMAX:2 * NMAX] = np.concatenate([mid, mid, mid], 1)
        eadd[s, :, 2 * NMAX:] = np.concatenate([mid, bot], 1)
    return wt, eadd


def kernel(x, extra_inputs, conv_w, conv_b, extra_w, extra_b):
    x = np.ascontiguousarray(np.asarray(x, np.float32))
    wt, eadd = host_prepack(
        np.asarray(extra_inputs, np.float32), np.asarray(conv_w, np.float32),
        np.asarray(conv_b, np.float32), np.asarray(extra_w, np.float32),
        np.asarray(extra_b, np.float32))

    nc = _get_program()
    in_maps = []
    for k in range(N_CORES):
        s0 = k * BL
        epair = np.stack(
            [np.concatenate([eadd[s0 + 2 * p], eadd[s0 + 2 * p + 1]], axis=0)
             for p in range(NPAIR)])
        in_maps.append({
            "x": x[s0:s0 + BL],
            "wt": wt,
            "eadd": np.ascontiguousarray(epair),
        })
    res = run_bass_kernel_spmd(nc, in_maps, list(range(N_CORES)))
    return np.concatenate([res.results[k]["out"] for k in range(N_CORES)],
                          axis=0)



# revision 2
# speedup vs baseline: 1.2244x; 1.2244x over previous
"""Bass/Trainium2 kernel for nn_CustomConvWithExtra (v5: RB=3 bf16).

Device math (per core, 4 samples as 2 sample-pairs):
  dense 3x3 conv as 9 shifted bf16 matmuls per band accumulating in
  fp32 PSUM; the "extra" path + biases are host-folded into a
  positional fp32 add-map applied in the PSUM->SBUF epilogue.

v4 changes vs v2 (206us):
  * x is PRE-PADDED AND PRE-CONVERTED on the host into the exact SBUF
    layout [NPAIR, 128(=2 samples x 64ch), XFREE] in bf16: zero halos,
    pad columns, everything.  Loads are fully contiguous per partition
    and run in 4 row-range chunks so group-0 matmuls start after ~1/4
    of the pair's data lands (v2 burned ~45us of fill on strided 512B-
    descriptor loads + device memsets).
  * bf16 matmuls (1 cycle/row like f32r, but no even-size/alignment
    restrictions, half the DMA bytes and half the SBUF footprint).

Layout: padded rows of stride 129 = [128 cols][1 zero pad] with a zero
halo row above/below and one leading zero; x[row, col] sits at
1 + (row+1)*129 + col.  Tap (di, dj) of the band at row i0 is the
contiguous window at (i0+di)*129 + dj.

Output goes out as [NPAIR, 128, H*W] fp32 (one 8KB-contiguous store per
8-band group); the host reorders to [B, COUT, H, W].
"""

import os

import numpy as np
import ml_dtypes

import concourse.bass as bass
import concourse.mybir as mybir
from concourse.tile import TileContext
from concourse.bass_utils import run_bass_kernel_spmd

N_CORES = 8
B, CIN, COUT, FES, H, W, KK = 32, 64, 64, 3, 128, 128, 3
BL = B // N_CORES          # samples per core
NPAIR = BL // 2            # sample pairs per core
RSTRIDE = 129              # padded row stride (W + 1 pad col)
XFREE = 1 + (H + 2) * RSTRIDE + 3   # 16774: lead zero + 130 padded rows + tail
RB = 3                     # output rows per band (bf16 allows odd sizes)
NBAND = (H + RB - 1) // RB  # 43 bands; last band has 2 rows
NMAX = RB * RSTRIDE        # 387 fp32 <= 512 (one PSUM bank)
GRP = 8                    # bands per store/PSUM group (= PSUM banks)
# x-load chunk boundaries in padded-row units; group g's matmuls need
# padded rows < 24g+26, so group g waits only on chunks <= g
XROWS = [0, 26, 50, 74, 98, 122, H + 2]

# eadd free-dim offsets: band 0 -> first pattern, 1..41 -> mid, 42 -> last
_E_FIRST, _E_MID, _E_LAST = 0, NMAX, 2 * NMAX
EADD_FREE = 2 * NMAX + (H - RB * (NBAND - 1)) * RSTRIDE  # 387+387+258


def split_sync_waits(nc):
    """This toolchain's walrus accepts only ONE sync-wait per instruction.
    Hoist extra waits onto single-wait NoOps inserted just before, on the
    same engine (same queue => same semantics)."""
    for func in nc.m.functions:
        for block in func.blocks:
            out = []
            changed = False
            for inst in block.instructions:
                si = inst.sync_info
                waits = list(si.on_wait) if (si and si.on_wait) else []
                if len(waits) > 1:
                    changed = True
                    for k, w in enumerate(waits[:-1]):
                        nop = mybir.InstNoOp(
                            name=f"{inst.name}-sw{k}",
                            engine=inst.engine,
                            sync_info=mybir.SyncInfo(on_wait=[w], on_update=[]),
                            bass_nofuse=True,
                        )
                        nc.register_instruction(nop, overwrite=True)
                        out.append(nop)
                    inst.sync_info = mybir.SyncInfo(
                        on_wait=[waits[-1]], on_update=list(si.on_update or [])
                    )
                out.append(inst)
            if changed:
                block.instructions = out


def build_program():
    f32 = mybir.dt.float32
    bf16 = mybir.dt.bfloat16
    nc = bass.Bass("TRN2", target_bir_lowering=False, debug=False,
                   num_devices=N_CORES)
    xpad = nc.dram_tensor("xpad", [NPAIR, 128, XFREE], bf16,
                          kind="ExternalInput")
    wt = nc.dram_tensor("wt", [128, 9 * 128], bf16, kind="ExternalInput")
    eadd = nc.dram_tensor("eadd", [NPAIR, 128, EADD_FREE], f32,
                          kind="ExternalInput")
    # [pair, (half, cout), row*W]; host reorders to [BL, COUT, H, W]
    out = nc.dram_tensor("out", [NPAIR, 128, H * W], f32,
                         kind="ExternalOutput")

    groups = [(g0, min(g0 + GRP, NBAND)) for g0 in range(0, NBAND, GRP)]

    with TileContext(nc) as tc:
        with (
            tc.tile_pool(name="wp", bufs=1) as wp,
            tc.tile_pool(name="xp", bufs=2) as xp,
            tc.tile_pool(name="ep", bufs=2) as ep,
            tc.tile_pool(name="op", bufs=2) as op,
            tc.tile_pool(name="pp", bufs=1, space="PSUM") as pp,
        ):
            wt_sb = wp.tile([128, 9 * 128], bf16)
            nc.sync.dma_start(out=wt_sb[:], in_=wt[:])

            for sp in range(NPAIR):
                xt = xp.tile([128, XFREE], bf16, tag="xt")
                for c in range(len(XROWS) - 1):
                    f0 = 0 if c == 0 else 1 + XROWS[c] * RSTRIDE
                    f1 = XFREE if c == len(XROWS) - 2 else \
                        1 + XROWS[c + 1] * RSTRIDE
                    nc.sync.dma_start(out=xt[:, f0:f1],
                                      in_=xpad[sp, :, f0:f1])
                et = ep.tile([128, EADD_FREE], f32, tag="et")
                nc.sync.dma_start(out=et[:], in_=eadd[sp])

                for g0, g1 in groups:
                    nb = g1 - g0
                    r0 = g0 * RB
                    r1 = min(g1 * RB, H)
                    ps = [pp.tile([128, NMAX], f32, tag=f"ps{k}",
                                  name=f"ps{k}")
                          for k in range(nb)]
                    for tap in range(9):
                        di, dj = divmod(tap, 3)
                        lhsT = wt_sb[:, tap * 128:(tap + 1) * 128]
                        st, sp_ = (tap == 0), (tap == 8)
                        for k in range(nb):
                            i0 = (g0 + k) * RB
                            rb = min(RB, H - i0)
                            n = rb * RSTRIDE
                            off = (i0 + di) * RSTRIDE + dj
                            nc.tensor.matmul(
                                ps[k][:, 0:n], lhsT,
                                xt[:, off:off + n],
                                start=st, stop=sp_)
                    ot = op.tile([128, (r1 - r0) * W], f32, tag="ot")
                    for k in range(nb):
                        i0 = (g0 + k) * RB
                        rb = min(RB, H - i0)
                        n = rb * RSTRIDE
                        eo = _E_FIRST if g0 + k == 0 else (
                            _E_LAST if g0 + k == NBAND - 1 else _E_MID)
                        dst = ot[:, (i0 - r0) * W:(i0 - r0 + rb) * W].rearrange(
                            "p (r c) -> p r c", c=W)
                        nc.vector.tensor_add(
                            dst,
                            ps[k][:, 0:n].rearrange(
                                "p (r c) -> p r c", c=RSTRIDE)[:, :, 0:W],
                            et[:, eo:eo + n].rearrange(
                                "p (r c) -> p r c", c=RSTRIDE)[:, :, 0:W])
                    nc.sync.dma_start(out=out[sp, :, r0 * W:r1 * W],
                                      in_=ot[:])

    split_sync_waits(nc)
    return nc


_PROGRAM = None
LAST_RESULT = None


def _get_program():
    global _PROGRAM
    if _PROGRAM is None:
        _PROGRAM = build_program()
    return _PROGRAM


def host_prepack(x, extra_inputs, conv_w, conv_b, extra_w, extra_b):
    """Fold weights/biases/extra-path into device-ready arrays and
    pre-pad x into the SBUF layout (bf16)."""
    bf16 = ml_dtypes.bfloat16
    # xpad[pair, half*64+ci, 1+(r+1)*129+c] = x[2*pair+half, ci, r, c]
    npair_g = B // 2
    xpad = np.zeros((npair_g, 128, XFREE), bf16)
    inner = np.zeros((npair_g, 128, H, RSTRIDE), bf16)
    inner[:, :, :, 0:W] = x.reshape(npair_g, 128, H, W)
    xpad[:, :, 1 + RSTRIDE:1 + (H + 1) * RSTRIDE] = inner.reshape(
        npair_g, 128, H * RSTRIDE)

    # block-diagonal weights: wt[ci, tap*128 + co] = conv_w[co, ci, di, dj]
    wt_half = np.ascontiguousarray(
        conv_w.transpose(1, 2, 3, 0)).reshape(CIN, 9, COUT)
    wt = np.zeros((128, 9, 128), np.float32)
    wt[0:64, :, 0:64] = wt_half
    wt[64:128, :, 64:128] = wt_half
    wt = wt.reshape(128, 9 * 128).astype(bf16)

    # border-case extra values: E[s, rowclass, colclass, co]
    row_sel = [slice(1, 3), slice(0, 3), slice(0, 2)]   # top, mid, bot
    col_sel = [slice(1, 3), slice(0, 3), slice(0, 2)]   # left, mid, right
    wsum = np.zeros((3, 3, COUT, FES), np.float32)
    for rc in range(3):
        for cc in range(3):
            wsum[rc, cc] = extra_w[:, :, row_sel[rc], col_sel[cc]].sum((2, 3))
    ein = extra_inputs.reshape(B, COUT, FES)
    e9 = np.einsum('scf,rkcf->srkc', ein, wsum)
    e9 = e9 + (extra_b + conv_b)[None, None, None, :]   # [s, rc, cc, co]

    # positional row patterns at stride 129 (last slot = pad, value 0)
    def row_vec(s, rc):
        v = np.zeros((COUT, RSTRIDE), np.float32)
        v[:, 0] = e9[s, rc, 0]
        v[:, 1:W - 1] = e9[s, rc, 1][:, None]
        v[:, W - 1] = e9[s, rc, 2]
        return v

    eadd = np.zeros((B, COUT, EADD_FREE), np.float32)
    for s in range(B):
        top, mid, bot = row_vec(s, 0), row_vec(s, 1), row_vec(s, 2)
        eadd[s, :, 0:NMAX] = np.concatenate([top, mid, mid], 1)
        eadd[s, :, NMAX:2 * NMAX] = np.concatenate([mid, mid, mid], 1)
        eadd[s, :, 2 * NMAX:] = np.concatenate([mid, bot], 1)
    return xpad, wt, eadd


def kernel(x, extra_inputs, conv_w, conv_b, extra_w, extra_b):
    global LAST_RESULT
    x = np.ascontiguousarray(np.asarray(x, np.float32))
    xpad, wt, eadd = host_prepack(
        x, np.asarray(extra_inputs, np.float32), np.asarray(conv_w, np.float32),
        np.asarray(conv_b, np.float32), np.asarray(extra_w, np.float32),
        np.asarray(extra_b, np.float32))

    nc = _get_program()
    in_maps = []
    for k in range(N_CORES):
        s0 = k * BL
        epair = np.stack(
            [np.concatenate([eadd[s0 + 2 * p], eadd[s0 + 2 * p + 1]], axis=0)
             for p in range(NPAIR)])
        in_maps.append({
            "xpad": xpad[k * NPAIR:(k + 1) * NPAIR],
            "wt": wt,
            "eadd": np.ascontiguousarray(epair),
        })
    res = run_bass_kernel_spmd(
        nc, in_maps, list(range(N_CORES)),
        trace=os.environ.get("KBENCH_TRACE", "") == "1")
    LAST_RESULT = res
    return np.concatenate(
        [res.results[k]["out"].reshape(BL, COUT, H, W) for k in range(N_CORES)],
        axis=0)
